# revision 4
# baseline (speedup 1.0000x reference)
"""MetaGAT Trainium2 kernel v8 (8 NeuronCores, SPMD).

Strategy (edge-parallel, batch-filtered, host pre-gathered, side-merged):
  Each core takes a 2048-slice of the batch; only edges whose destination
  is in that slice matter (~20K of 2M per side).  The host does pure data
  movement: selects those edges, degree-buckets destinations into
  [128-lane x window] slot grids (16 windows per side, shared per-window
  capacity schedule KS), and materializes the source-embedding stream in
  grid order, d-major per window ([65, k] blocks: 64 emb dims pre-scaled
  by wa_src + an all-ones "den" feature), cast to bf16.  The u/i sides
  are interleaved window-by-window (they share KS) and consecutive
  equal-k windows are merged into runs (DP-chosen buckets), so the device
  processes ~8 runs of [128, n, 65, k] blocks with one instruction per
  stage per run:
    - edge scores: packed bf16 add-tree over d + one strided reduce
      (features are pre-scaled by wa_src; dst scores are a plain reduce
      of the wa_dst-pre-scaled dstT; 1/wa_src is folded into Wn)
    - leaky-relu + exp (bf16), then weighted sums via one mult + k-fold +
      packed reduce; the den column yields the softmax denominator and
      self-masks padding
    - normalize, PE-transpose per window, and the 3-linear MLP per run
  Everything streams: one DMA per run, consumed and released in order.
  Outputs are [64, 2048] grid-slot columns per side; the host maps grid
  slots back to batch rows (pure indexing) and assembles [16384, 128].
"""
import numpy as np
import ml_dtypes

BF16 = ml_dtypes.bfloat16
EMB = 64
D = EMB                   # stream width: 64 emb dims (pads carry -1e30 tag)
NNODE = 200000
NCORES = 8
B = 16384
BC = B // NCORES          # 2048 batch rows per core
GPAD = BC                 # grid slots per side (>= unique dst count)
NW = GPAD // 128          # 16 windows of 128 dst lanes per side
NWC = 2 * NW              # combined (side-interleaved) window count
F32MIN = 1e-30


# ----------------------------------------------------------------- host prep

def _prep_core_side(bat_c, src_ids, dst_ids):
    """Pure index bookkeeping for one (core, side): select + grid-order edges."""
    uniq, inv = np.unique(bat_c, return_inverse=True)
    G = uniq.size
    lut = np.full(NNODE, -1, np.int32)
    lut[uniq] = np.arange(G, dtype=np.int32)
    eg = lut[dst_ids]
    m = eg >= 0
    es = src_ids[m].astype(np.int64)
    eg = eg[m].astype(np.int64)
    deg = np.bincount(eg, minlength=G)
    order = np.argsort(-deg, kind="stable")          # grid rank -> uniq idx
    pos = np.empty(G, np.int64)
    pos[order] = np.arange(G)
    deg_r = deg[order]                               # degree by rank (desc)
    ep = pos[eg]                                     # edge -> grid rank
    eo = np.argsort(ep, kind="stable")
    es_s = es[eo]
    ep_s = ep[eo]
    starts = np.zeros(G + 1, np.int64)
    np.cumsum(deg_r, out=starts[1:])
    ii = np.arange(es_s.size) - starts[ep_s]         # slot index within dst
    bslot = pos[inv]                                 # batch row -> grid rank
    return dict(G=G, uniq=uniq, order=order, deg_r=deg_r,
                es_s=es_s, ep_s=ep_s, ii=ii, bslot=bslot)


def _runs_from_ks(KS):
    """DP-partition the combined window sequence into equal-k runs.

    Units are side-PAIRS (2 combined windows) with capacity KS[j]; a run
    [i..j] costs 2*(j-i+1)*KS[i] slot-columns (KS descending) + LAM fixed.
    """
    LAM = 5.0
    n = len(KS)
    best = [None] * (n + 1)
    best[n] = (0.0, [])
    for i in range(n - 1, -1, -1):
        cands = []
        for j in range(i, n):
            c = 2.0 * (j - i + 1) * KS[i] + LAM + best[j + 1][0]
            cands.append((c, [(2 * i, 2 * (j - i + 1), KS[i])] + best[j + 1][1]))
        best[i] = min(cands, key=lambda t: t[0])
    return best[0][1]                                # [(w0_combined, n, k)]


def _stream_side(pc, KSB, emb_src, wa_src):
    """Per-window [128, D*k] d-major blocks (bf16), source pre-scaled."""
    KMAX = max(KSB)
    idx_mat = np.full((GPAD, KMAX), -1, np.int64)
    idx_mat[pc["ep_s"], pc["ii"]] = pc["es_s"]
    blocks = []
    for w in range(NW):
        k = KSB[w]
        sub = idx_mat[w * 128:(w + 1) * 128, :k]          # [128, k]
        msk = sub >= 0
        feats = emb_src[sub.clip(0)] * wa_src[None, None, :]
        feats[~msk] = 0.0
        # pad slots get a -1e30 tag in dim 0: their edge score folds to
        # -1e30, exp gives exactly 0 (self-masking without a mask column)
        feats[:, :, 0] = np.where(msk, feats[:, :, 0], -1e30)
        blocks.append(np.ascontiguousarray(
            feats.transpose(0, 2, 1).reshape(128, D * k)).astype(BF16))
    return blocks


def _prep_all(inputs):
    u = np.asarray(inputs["u"]).astype(np.int64)
    i_ = np.asarray(inputs["i"]).astype(np.int64)
    sides = {
        "u": dict(bat=u, src=np.asarray(inputs["src_iu"]).astype(np.int64),
                  dst=np.asarray(inputs["dst_iu"]).astype(np.int64),
                  emb_src=np.asarray(inputs["item_emb"], np.float32),
                  emb_dst=np.asarray(inputs["user_emb"], np.float32)),
        "i": dict(bat=i_, src=np.asarray(inputs["src_ui"]).astype(np.int64),
                  dst=np.asarray(inputs["dst_ui"]).astype(np.int64),
                  emb_src=np.asarray(inputs["user_emb"], np.float32),
                  emb_dst=np.asarray(inputs["item_emb"], np.float32)),
    }
    pcs = {s: [_prep_core_side(sides[s]["bat"][c * BC:(c + 1) * BC],
                               sides[s]["src"], sides[s]["dst"])
               for c in range(NCORES)] for s in sides}
    # shared per-window capacity schedule across cores & sides
    KS = []
    for w in range(NW):
        k = 1
        for s in pcs:
            for pc in pcs[s]:
                if w * 128 < pc["G"]:
                    k = max(k, int(pc["deg_r"][w * 128]))
        KS.append(k)
    RUNS = _runs_from_ks(KS)                         # combined-window runs
    KSB = []                                         # bucketed per-side-window k
    for (w0, n, k) in RUNS:
        KSB.extend([k] * (n // 2))
    assert len(KSB) == NW and all(a >= b for a, b in zip(KSB, KS))

    was = {s: np.asarray(inputs[f"Wa_{s}"], np.float32) for s in ("u", "i")}
    per_core = []
    bslots = []
    for c in range(NCORES):
        blocks_u = _stream_side(pcs["u"][c], KSB, sides["u"]["emb_src"],
                                was["u"][:EMB])
        blocks_i = _stream_side(pcs["i"][c], KSB, sides["i"]["emb_src"],
                                was["i"][:EMB])
        inter = []
        for w in range(NW):
            inter.append(blocks_u[w])
            inter.append(blocks_i[w])
        Fd = np.ascontiguousarray(np.concatenate(inter, axis=1))
        d = {"Fd": Fd}
        bs = {}
        # combined dstT (side-interleaved windows), pre-scaled by wa_dst;
        # per-side embT (unscaled, feature-major) for the MLP
        grids = {}
        for s in ("u", "i"):
            pc = pcs[s][c]
            grid = np.zeros((GPAD, EMB), np.float32)
            grid[:pc["G"]] = sides[s]["emb_dst"][pc["uniq"][pc["order"]]]
            grids[s] = grid
            d[f"embT_{s}"] = np.ascontiguousarray(grid.T).astype(BF16)
            bs[s] = pc["bslot"]
        dstc = np.empty((128, NWC * EMB), np.float32)
        for w in range(NW):
            for si, s in enumerate(("u", "i")):
                gw = grids[s][w * 128:(w + 1) * 128] * was[s][EMB:][None, :]
                dstc[:, (2 * w + si) * EMB:(2 * w + si + 1) * EMB] = gw
        d["dstT"] = dstc.astype(BF16)
        per_core.append(d)
        bslots.append(bs)
    # weights (identical on every core)
    w = {}
    for s, Ws, bs_, Wn, bn, Wfc in (
            ("u", inputs["Ws_u"], inputs["bs_u"],
             inputs["Wn_u"], inputs["bn_u"], inputs["Wfc_u"]),
            ("i", inputs["Ws_i"], inputs["bs_i"],
             inputs["Wn_i"], inputs["bn_i"], inputs["Wfc_i"])):
        wa_src = was[s][:EMB].copy()
        wa_src[np.abs(wa_src) < 1e-25] = 1e-25       # guard 1/wa_src
        w[f"WsT_{s}"] = np.ascontiguousarray(np.asarray(Ws, np.float32).T).astype(BF16)
        WnT = np.ascontiguousarray(np.asarray(Wn, np.float32).T)
        w[f"WnT_{s}"] = (WnT / wa_src[:, None]).astype(BF16)
        Wfc = np.asarray(Wfc, np.float32)
        w[f"WfcS_{s}"] = np.ascontiguousarray(Wfc[:, :EMB].T).astype(BF16)
        w[f"WfcN_{s}"] = np.ascontiguousarray(Wfc[:, EMB:].T).astype(BF16)
        w[f"bs_{s}"] = np.asarray(bs_, np.float32).reshape(EMB, 1)
        w[f"bn_{s}"] = np.asarray(bn, np.float32).reshape(EMB, 1)
    for d in per_core:
        d.update(w)
    cfg = dict(RUNS=tuple(RUNS), KSB=tuple(KSB))
    return cfg, per_core, bslots


# ------------------------------------------------------------- device kernel

def _build_nc(cfg):
    import concourse.bacc as bacc
    import concourse.mybir as mybir
    import concourse.tile as tile
    from concourse.masks import make_identity

    f32 = mybir.dt.float32
    bf16 = mybir.dt.bfloat16
    RUNS = cfg["RUNS"]
    KSB = cfg["KSB"]
    SLC = sum(n * k for (_, n, k) in RUNS)           # combined slot-columns
    Alu = mybir.AluOpType
    Act = mybir.ActivationFunctionType

    nc = bacc.Bacc("TRN2")
    T = {}
    T["Fd"] = nc.dram_tensor("Fd", [128, D * SLC], bf16, kind="ExternalInput")
    T["dstT"] = nc.dram_tensor("dstT", [128, NWC * EMB], bf16, kind="ExternalInput")
    for s in ("u", "i"):
        T[f"embT_{s}"] = nc.dram_tensor(f"embT_{s}", [EMB, GPAD], bf16, kind="ExternalInput")
        for nm in ("WsT", "WnT", "WfcS", "WfcN"):
            T[f"{nm}_{s}"] = nc.dram_tensor(f"{nm}_{s}", [EMB, EMB], bf16, kind="ExternalInput")
        for nm in ("bs", "bn"):
            T[f"{nm}_{s}"] = nc.dram_tensor(f"{nm}_{s}", [EMB, 1], f32, kind="ExternalInput")
        T[f"outT_{s}"] = nc.dram_tensor(f"outT_{s}", [EMB, GPAD], f32, kind="ExternalOutput")

    with tile.TileContext(nc) as tc:
        with (
            tc.tile_pool(name="gpool", bufs=1) as gpool,
            tc.tile_pool(name="fpool", bufs=3) as fpool,
            tc.tile_pool(name="wpool", bufs=3) as wpool,
            tc.tile_pool(name="spool", bufs=3) as spool,
            tc.tile_pool(name="cpool", bufs=1) as cpool,
            tc.tile_pool(name="ptp", bufs=2, space="PSUM") as ptp,
            tc.tile_pool(name="pmm", bufs=2, space="PSUM") as pmm,
        ):
            ident = cpool.tile([128, 128], bf16)
            make_identity(nc, ident[:])

            # ---- global loads on the Act HWDGE queue (runs stream on SP)
            dstT = gpool.tile([128, NWC, EMB], bf16, tag="dstT")
            nc.scalar.dma_start(
                dstT[:], T["dstT"][:].rearrange("p (n d) -> p n d", d=EMB))
            wsm = {}
            for s in ("u", "i"):
                for nm in ("WsT", "WnT", "WfcS", "WfcN"):
                    t = gpool.tile([EMB, EMB], bf16, tag=f"{nm}{s}", name=nm)
                    nc.scalar.dma_start(t[:], T[f"{nm}_{s}"][:])
                    wsm[f"{nm}_{s}"] = t
                for nm in ("bs", "bn"):
                    t = gpool.tile([EMB, 1], f32, tag=f"{nm}{s}", name=nm)
                    nc.scalar.dma_start(t[:], T[f"{nm}_{s}"][:])
                    wsm[f"{nm}_{s}"] = t
            embT = {}
            for s in ("u", "i"):
                embT[s] = gpool.tile([EMB, GPAD], bf16, tag=f"embT{s}",
                                     name="embT")
                nc.scalar.dma_start(embT[s][:], T[f"embT_{s}"][:])
            hT = {s: gpool.tile([EMB, GPAD], bf16, tag=f"hT{s}", name="hT")
                  for s in ("u", "i")}

            # ---- streamed runs
            sdst = None
            cw = 0                                    # combined slot offset
            for ri, (w0, n, k) in enumerate(RUNS):
                FR = fpool.tile([128, n * D * k], bf16, tag="FR", name="FR")
                nc.sync.dma_start(
                    FR[:], T["Fd"][:, D * cw:D * (cw + n * k)])
                cw += n * k
                frv = FR[:].rearrange("p (n d k) -> p n d k", n=n, d=D)

                # scores: packed d-fold 64 -> 8, then strided reduce over 8
                a1 = wpool.tile([128, n * 32 * k], bf16, tag="a1", name="a1")
                a1v = a1[:].rearrange("p (n d k) -> p n d k", n=n, d=32)
                nc.vector.tensor_tensor(
                    out=a1v, in0=frv[:, :, :32, :], in1=frv[:, :, 32:EMB, :],
                    op=Alu.add)
                a2 = wpool.tile([128, n * 16 * k], bf16, tag="a2", name="a2")
                a2v = a2[:].rearrange("p (n d k) -> p n d k", n=n, d=16)
                nc.vector.tensor_tensor(
                    out=a2v, in0=a1v[:, :, :16, :], in1=a1v[:, :, 16:, :],
                    op=Alu.add)
                a3 = wpool.tile([128, n * 8 * k], bf16, tag="a3", name="a3")
                a3v = a3[:].rearrange("p (n d k) -> p n d k", n=n, d=8)
                nc.vector.tensor_tensor(
                    out=a3v, in0=a2v[:, :, :8, :], in1=a2v[:, :, 8:, :],
                    op=Alu.add)
                ss = wpool.tile([128, n * k], bf16, tag="ss", name="ss")
                with nc.allow_low_precision(reason="bf16 scores"):
                    nc.vector.tensor_reduce(
                        out=ss[:].rearrange("p (n k) -> p n k", n=n),
                        in_=a3[:].rearrange("p (n d k) -> p n k d", n=n, d=8),
                        axis=mybir.AxisListType.X, op=Alu.add)

                # s_dst for all combined windows (pre-scaled by wa_dst);
                # emitted here so run-0 scores aren't queued behind the
                # dstT DMA on the in-order DVE sequencer
                if sdst is None:
                    sdst = gpool.tile([128, NWC], bf16, tag="sdst")
                    with nc.allow_low_precision(reason="bf16 scores"):
                        nc.vector.tensor_reduce(
                            out=sdst[:], in_=dstT[:],
                            axis=mybir.AxisListType.X, op=Alu.add)

                # e = lrelu(ss + sdst); exm = exp(e)
                e_ = wpool.tile([128, n * k], bf16, tag="e", name="e_")
                nc.vector.tensor_tensor(
                    out=e_[:].rearrange("p (n k) -> p n k", n=n),
                    in0=ss[:].rearrange("p (n k) -> p n k", n=n),
                    in1=sdst[:, w0:w0 + n].unsqueeze(2).to_broadcast([128, n, k]),
                    op=Alu.add)
                lr = wpool.tile([128, n * k], bf16, tag="lr", name="lr")
                nc.vector.scalar_tensor_tensor(
                    out=lr[:], in0=e_[:], scalar=0.01, in1=e_[:],
                    op0=Alu.mult, op1=Alu.max)
                exm = wpool.tile([128, n * k], bf16, tag="exm", name="exm")
                nc.scalar.activation(exm[:], lr[:], Act.Exp)

                # den = sum_k exm (pads are exactly 0); on all but the last
                # run pre-scale exm by 1/den so the weighted sum yields h
                # directly.  The last run post-scales instead, so den/recip
                # run parallel to its weighted sum (shorter tail chain).
                last = False
                den = spool.tile([128, n], f32, tag="den", name="den")
                nc.vector.tensor_reduce(
                    out=den[:], in_=exm[:].rearrange("p (n k) -> p n k", n=n),
                    axis=mybir.AxisListType.X, op=Alu.add)
                nc.vector.tensor_scalar_max(out=den[:], in0=den[:],
                                            scalar1=F32MIN)
                invd = spool.tile([128, n], bf16, tag="invd", name="invd")
                with nc.allow_low_precision(reason="bf16 inv-den"):
                    nc.vector.reciprocal(invd[:], den[:])
                if not last:
                    exn = wpool.tile([128, n * k], bf16, tag="exn", name="exn")
                    nc.vector.tensor_tensor(
                        out=exn[:].rearrange("p (n k) -> p n k", n=n),
                        in0=exm[:].rearrange("p (n k) -> p n k", n=n),
                        in1=invd[:].unsqueeze(2).to_broadcast([128, n, k]),
                        op=Alu.mult)
                else:
                    exn = exm

                # weighted sums with (pre-)normalized alpha
                fw2 = wpool.tile([128, n * D * k], bf16, tag="fw2", name="fw2")
                f2v = fw2[:].rearrange("p (n d k) -> p n d k", n=n, d=D)
                nc.vector.tensor_tensor(
                    out=f2v, in0=frv,
                    in1=exn[:].rearrange("p (n k) -> p n k", n=n)
                        .unsqueeze(2).to_broadcast([128, n, D, k]),
                    op=Alu.mult)
                # k-fold tree (packed bf16) down to <= 4, then reduce
                src = f2v
                kk = k
                while kk > 4:
                    m = kk // 2
                    b = wpool.tile([128, n * D * m], bf16, tag=f"b{m}",
                                   name="bfold")
                    bv = b[:].rearrange("p (n d k) -> p n d k", n=n, d=D)
                    nc.vector.tensor_tensor(
                        out=bv, in0=src[:, :, :, :m], in1=src[:, :, :, m:2 * m],
                        op=Alu.add)
                    if kk % 2:
                        nc.vector.tensor_tensor(
                            out=bv[:, :, :, 0:1], in0=bv[:, :, :, 0:1],
                            in1=src[:, :, :, kk - 1:kk], op=Alu.add)
                    src = bv
                    kk = m
                hg = spool.tile([128, n * D], bf16, tag="hg", name="hg")
                with nc.allow_low_precision(reason="f32 accum, bf16 out"):
                    nc.vector.tensor_reduce(
                        out=hg[:],
                        in_=src.rearrange("p n d k -> p (n d) k"),
                        axis=mybir.AxisListType.X, op=Alu.add)

                hgv = hg[:].rearrange("p (n d) -> p n d", n=n)
                if last:
                    hn = spool.tile([128, n, EMB], bf16, tag="hn", name="hn")
                    nc.vector.tensor_tensor(
                        out=hn[:], in0=hgv,
                        in1=invd[:].unsqueeze(2).to_broadcast([128, n, EMB]),
                        op=Alu.mult)
                    hgv = hn[:]

                # transpose per side (combined window w0+j: even=u, odd=i)
                for par, s in ((0, "u"), (1, "i")):
                    js = [j for j in range(n) if (w0 + j) % 2 == par]
                    for g in range(0, len(js), 4):
                        grp = js[g:g + 4]
                        pt = ptp.tile([EMB, 512], bf16, tag="pt")
                        for gi, j in enumerate(grp):
                            nc.tensor.transpose(
                                pt[:, gi * 128:(gi + 1) * 128],
                                hgv[:, j, :], ident[:])
                        sw = (w0 + grp[0]) // 2
                        nc.scalar.copy(
                            out=hT[s][:, sw * 128:sw * 128 + len(grp) * 128],
                            in_=pt[:, :len(grp) * 128])

                    # MLP for this run's columns of side s (narrow chunks on
                    # the final run to shorten the tail dependency chain)
                    if not js:
                        continue
                    wid = 512
                    sw0 = (w0 + js[0]) // 2
                    c0_, c1_ = sw0 * 128, (sw0 + len(js)) * 128
                    while c0_ < c1_:
                        CHK = min(wid, c1_ - c0_)
                        sl_ = slice(c0_, c0_ + CHK)
                        c0_ += CHK
                        psf = pmm.tile([EMB, 512], f32, tag="psf")
                        nc.tensor.matmul(psf[:, :CHK], wsm[f"WsT_{s}"][:],
                                         embT[s][:, sl_], start=True, stop=True)
                        pnb = pmm.tile([EMB, 512], f32, tag="pnb")
                        nc.tensor.matmul(pnb[:, :CHK], wsm[f"WnT_{s}"][:],
                                         hT[s][:, sl_], start=True, stop=True)
                        sfr = spool.tile([EMB, 512], bf16, tag="sfr", name="sfr")
                        nc.scalar.activation(sfr[:, :CHK], psf[:, :CHK],
                                             Act.Relu, bias=wsm[f"bs_{s}"][:])
                        nbr = spool.tile([EMB, 512], bf16, tag="nbr", name="nbr")
                        nc.scalar.activation(nbr[:, :CHK], pnb[:, :CHK],
                                             Act.Relu, bias=wsm[f"bn_{s}"][:])
                        pv = pmm.tile([EMB, 512], f32, tag="pv")
                        nc.tensor.matmul(pv[:, :CHK], wsm[f"WfcS_{s}"][:],
                                         sfr[:, :CHK], start=True, stop=False)
                        nc.tensor.matmul(pv[:, :CHK], wsm[f"WfcN_{s}"][:],
                                         nbr[:, :CHK], start=False, stop=True)
                        ov = spool.tile([EMB, 512], f32, tag="ov", name="ov")
                        nc.scalar.activation(ov[:, :CHK], pv[:, :CHK], Act.Relu)
                        nc.sync.dma_start(T[f"outT_{s}"][:, sl_], ov[:, :CHK])

    nc.compile()
    return nc


# ------------------------------------------------------------------ assembly

def _assemble(results, bslots):
    out = np.empty((B, 2 * EMB), np.float32)
    for c, r in enumerate(results):
        out[c * BC:(c + 1) * BC, :EMB] = r["outT_u"].T[bslots[c]["u"]]
        out[c * BC:(c + 1) * BC, EMB:] = r["outT_i"].T[bslots[c]["i"]]
    return out


def build_all(inputs):
    cfg, per_core, bslots = _prep_all(inputs)
    nc = _build_nc(cfg)
    return nc, per_core, bslots


def kernel(**inputs) -> np.ndarray:
    from concourse.bass_utils import run_bass_kernel_spmd
    nc, per_core, bslots = build_all(inputs)
    res = run_bass_kernel_spmd(nc, per_core, core_ids=list(range(NCORES)))
    return _assemble(res.results, bslots)
